# revision 1
# baseline (speedup 1.0000x reference)
"""2-layer GAT (gnn_message_passing) on 8 Trainium2 NeuronCores.

Strategy (per sharding hint): nodes are partitioned contiguously across the 8
cores (12500 each). Edges (incl. self-loops) are sharded by destination core,
sorted by destination window (128 dst nodes) and source range (quarter of the
node space, so gather indices fit int16), and padded to a static tile grid.

Per layer: a dense phase computes per-node transformed features
hp = x @ W and attention logit halves al_src/al_dst (folded into the weight
matrix as extra columns), writes them into a 256B-strided row table, and an
AllGather replicates the table to every core. The edge phase gathers rows by
edge source (custom dma_gather), gathers al_dst by edge destination, forms
ee = exp(leaky_relu(al_s + al_d)) per edge, multiplies messages, and
scatter-adds per destination window with a one-hot selection matmul on the PE
(which also accumulates the softmax denominators). Output rows are contiguous
per window, so no scatter is needed on the way out.
"""
import math
import numpy as np
import ml_dtypes

import concourse.bacc as bacc
import concourse.mybir as mybir
import concourse.tile as tile
from concourse import ap_utils

bf16 = ml_dtypes.bfloat16
F32 = mybir.dt.float32
BF16 = mybir.dt.bfloat16
I16 = mybir.dt.int16
I32 = mybir.dt.int32

P = 128
MAX_IDX_PER_GATHER = 3840   # DMA desc ring: <=~4080 idxs per gather inst
SLOPE = 0.2


# ---------------------------------------------------------------- dma_gather
def dma_gather_raw(eng, out_ap, in_ap, idxs_ap, num_idxs, elem_size,
                   elem_step=None, queue_num=0):
    """BassGpSimd.dma_gather (DRAM src, non-transpose) minus the
    elem_size%256B assert (transpose-only restriction, see q7 source) and
    with single_packet=False (large single packets wedge the SDMA)."""
    assert idxs_ap.dtype == mybir.dt.int16
    assert in_ap.dtype == out_ap.dtype
    elem_size_bytes = elem_size * mybir.dt.size(in_ap.dtype)
    assert elem_size_bytes > 0
    if elem_step is None:
        elem_step = elem_size
    assert ap_utils.ap_is_contiguous(in_ap.ap[1:])
    assert ap_utils.ap_is_contiguous(out_ap.ap[1:])
    assert ap_utils.ap_is_contiguous(idxs_ap.ap[1:])
    assert in_ap.ap[0][0] == elem_step
    assert in_ap.ap[-1][1] == elem_size
    assert out_ap.ap[-1][1] == elem_size
    assert num_idxs <= MAX_IDX_PER_GATHER + 256
    stride_bytes = elem_step * mybir.dt.size(in_ap.dtype)
    assert stride_bytes % 256 == 0 and stride_bytes // 256 < 256
    _in_ap = eng.lower_ap_dma(in_ap, for_custom_bir_dma=True)
    _idxs_ap = eng.lower_ap(idxs_ap)
    _out_ap = eng.lower_ap(out_ap)
    return eng.add_instruction(
        mybir.InstDMAGatherAnt(
            name=eng.bass.get_next_instruction_name(),
            ins=[*_in_ap, _idxs_ap, eng.lower_val_access(eng.to_reg(num_idxs))],
            outs=[_out_ap],
            transpose=False,
            num_idxs=num_idxs,
            elem_size=elem_size,
            stride_bytes_256=stride_bytes // 256,
            gen_mode=0,
            single_packet=False,
            queue_num=queue_num,
            sbuf_tokens_per_rank=0,
            sbuf_free_dim_per_rank=0,
            sbuf_free_dim_pad_per_rank=0,
            sbuf_byte_offset=0,
        )
    )


# ------------------------------------------------------------- host preproc
def _wrap_idx(seq16):
    """[NWIN, L] int -> [NWIN, 128, L//16] int16 in dma_gather idx layout
    (idx j at lane j%16 col j//16, replicated to 8 lane groups)."""
    nw, L = seq16.shape
    w = seq16.reshape(nw, L // 16, 16).transpose(0, 2, 1)      # [NWIN,16,K]
    w = np.tile(w, (1, 8, 1))                                  # [NWIN,128,K]
    return np.ascontiguousarray(w.astype(np.int16))


def preprocess(edge_index, cfg):
    """Sort/pad edges into the static (core, window, range, tile, lane) grid."""
    N, ncores, nloc, nwin, nrange = (cfg["N"], cfg["ncores"], cfg["nloc"],
                                     cfg["nwin"], cfg["nrange"])
    rng_sz = N // nrange
    loops = np.arange(N, dtype=np.int64)
    src = np.concatenate([edge_index[0].astype(np.int64), loops])
    dst = np.concatenate([edge_index[1].astype(np.int64), loops])
    core = dst // nloc
    dst_loc = dst - core * nloc
    w = dst_loc // P
    dst_rel = dst_loc - w * P
    r = src // rng_sz
    src_rel = src - r * rng_sz
    key = (core * nwin + w) * nrange + r
    counts = np.bincount(key, minlength=ncores * nwin * nrange)
    t_r = max(1, math.ceil(counts.max() / P))
    run = t_r * P
    order = np.argsort(key, kind="stable")
    ks = key[order]
    starts = np.zeros(ncores * nwin * nrange + 1, np.int64)
    np.cumsum(counts, out=starts[1:])
    pos = np.arange(len(ks)) - starts[ks]
    slot = ks * run + pos
    tot = ncores * nwin * nrange * run
    srcrel_f = np.zeros(tot, np.int16)
    dstrel_f = np.full(tot, -1.0, np.float32)
    dstloc_f = np.zeros(tot, np.int16)
    srcrel_f[slot] = src_rel[order].astype(np.int16)
    dstrel_f[slot] = dst_rel[order].astype(np.float32)
    dstloc_f[slot] = dst_loc[order].astype(np.int16)
    srcrel_f = srcrel_f.reshape(ncores, nwin, nrange, run)
    dstrel_f = dstrel_f.reshape(ncores, nwin, nrange, t_r, P)
    dstloc_f = dstloc_f.reshape(ncores, nwin, nrange, run)

    per_core = []
    for c in range(ncores):
        m = {}
        for rr in range(nrange):
            # [128, NWIN, K] idx layouts (K = t_r*8)
            m[f"iA{rr}"] = np.ascontiguousarray(
                _wrap_idx(srcrel_f[c, :, rr, :]).transpose(1, 0, 2))
            m[f"iB{rr}"] = np.ascontiguousarray(
                _wrap_idx(dstloc_f[c, :, rr, :]).transpose(1, 0, 2))
        # dstrel device layout [128, NWIN, nrange*t_r]
        m["dstrel"] = np.ascontiguousarray(
            dstrel_f[c].transpose(3, 0, 1, 2).reshape(P, nwin, nrange * t_r))
        per_core.append(m)
    return per_core, t_r


# ------------------------------------------------------------- device build
def build_nc(cfg, t_r):
    N, ncores, nloc, nwin, nrange = (cfg["N"], cfg["ncores"], cfg["nloc"],
                                     cfg["nwin"], cfg["nrange"])
    F_IN, H1, C1, C2 = cfg["F_IN"], cfg["H1"], cfg["C1"], cfg["C2"]
    D1 = H1 * C1                   # 64
    A1 = D1 + 2 * H1               # 80: [hp | al_s | al_d]
    T1W = D1 + H1                  # 72 table row (hp | al_s)
    D2 = C2                        # 16
    T2W = D2 + 2                   # 18 table row (hp2 | al_s2 | pad)
    K = t_r * 8                    # idx cols per window
    G = max(1, MAX_IDX_PER_GATHER // (t_r * P))     # windows per group
    n_groups = math.ceil(nwin / G)
    kchunks = F_IN // P
    last_rows = nloc - (nwin - 1) * P

    nc = bacc.Bacc("TRN2", target_bir_lowering=False, num_devices=ncores)
    xT = nc.dram_tensor("xT", [F_IN, nloc], BF16, kind="ExternalInput")
    W1e = nc.dram_tensor("W1e", [F_IN, A1], BF16, kind="ExternalInput")
    W2e = nc.dram_tensor("W2e", [D1, T2W], BF16, kind="ExternalInput")
    b1r = nc.dram_tensor("b1r", [P, D1], F32, kind="ExternalInput")
    b2r = nc.dram_tensor("b2r", [P, D2], F32, kind="ExternalInput")
    iA = [nc.dram_tensor(f"iA{rr}", [P, nwin, K], I16, kind="ExternalInput")
          for rr in range(nrange)]
    iB = [nc.dram_tensor(f"iB{rr}", [P, nwin, K], I16, kind="ExternalInput")
          for rr in range(nrange)]
    dstrel = nc.dram_tensor("dstrel", [P, nwin, nrange * t_r], F32,
                            kind="ExternalInput")
    out = nc.dram_tensor("out", [nloc, D2], F32, kind="ExternalOutput")

    SEL_W = nrange * t_r * P       # sel columns per window

    with tile.TileContext(nc) as tc:
        with (
            tc.tile_pool(name="const", bufs=1) as cpool,
            tc.tile_pool(name="sbuf", bufs=2) as sb,
            tc.tile_pool(name="gat", bufs=2) as gp,
            tc.tile_pool(name="psum", bufs=2, space="PSUM") as ps,
            tc.tile_pool(name="psum1", bufs=2, space="PSUM") as ps1,
            tc.tile_pool(name="dram", bufs=1, space="DRAM") as dr,
        ):
            t1loc = dr.tile([nloc, P], BF16)
            t1full = dr.tile([N, P], BF16)
            ald1 = dr.tile([nloc, 64], F32)
            t2loc = dr.tile([nloc, P], BF16)
            t2full = dr.tile([N, P], BF16)
            ald2 = dr.tile([nloc, 64], F32)

            # ---- static constants
            w1s = cpool.tile([P, kchunks, A1], BF16)
            nc.sync.dma_start(out=w1s[:], in_=W1e[:].rearrange(
                "(c p) a -> p c a", p=P))
            w2s = cpool.tile([D1, T2W], BF16)
            nc.sync.dma_start(out=w2s[:], in_=W2e[:])
            b1s = cpool.tile([P, D1], F32)
            nc.sync.dma_start(out=b1s[:], in_=b1r[:])
            b2s = cpool.tile([P, D2], F32)
            nc.sync.dma_start(out=b2s[:], in_=b2r[:])
            iota_i = cpool.tile([P, SEL_W], I32)
            nc.gpsimd.iota(iota_i[:], pattern=[[0, nrange * t_r], [1, P]],
                           base=0, channel_multiplier=0)
            iota_f = cpool.tile([P, SEL_W], F32)
            nc.vector.tensor_copy(out=iota_f[:], in_=iota_i[:])
            ident = cpool.tile([P, P], F32)
            from concourse.masks import make_identity
            make_identity(nc, ident[:])

            # ---- phase A: tables for layer 1
            for b in range(nwin):
                r0 = b * P
                rows = P if b < nwin - 1 else last_rows
                xa = sb.tile([P, kchunks, P], BF16, tag="xa")
                nc.sync.dma_start(
                    out=xa[:, :, :rows],
                    in_=xT[:, r0:r0 + rows].rearrange("(c p) r -> p c r", p=P))
                pA = ps.tile([P, A1], F32, tag="pA")
                for c in range(kchunks):
                    nc.tensor.matmul(pA[:], lhsT=xa[:, c, :], rhs=w1s[:, c, :],
                                     start=(c == 0), stop=(c == kchunks - 1))
                t1row = sb.tile([P, T1W], BF16, tag="t1row")
                nc.vector.tensor_copy(out=t1row[:], in_=pA[:, 0:T1W])
                a1row = sb.tile([P, H1], F32, tag="a1row")
                nc.scalar.copy(out=a1row[:], in_=pA[:, T1W:A1])
                nc.sync.dma_start(out=t1loc[r0:r0 + rows, 0:T1W],
                                  in_=t1row[:rows, :])
                nc.sync.dma_start(out=ald1[r0:r0 + rows, 0:H1],
                                  in_=a1row[:rows, :])

            # ---- allgather T1
            nc.gpsimd.collective_compute(
                "AllGather", mybir.AluOpType.bypass,
                replica_groups=[list(range(ncores))],
                ins=[t1loc[:].opt()], outs=[t1full[:].opt()])

            # ---- edge phases
            def edge_phase(layer):
                tfull = t1full if layer == 1 else t2full
                ald = ald1 if layer == 1 else ald2
                TW = T1W if layer == 1 else T2W
                DH = D1 if layer == 1 else D2       # message width
                NH = H1 if layer == 1 else 1        # heads
                CH = DH // NH
                rng_rows = N // nrange
                for g in range(n_groups):
                    w0 = g * G
                    Gg = min(G, nwin - w0)
                    nidx = Gg * t_r * P
                    hp_g, ald_g = [], []
                    for rr in range(nrange):
                        it = sb.tile([P, G, K], I16, tag=f"iA{rr}")
                        nc.sync.dma_start(out=it[:, :Gg, :],
                                          in_=iA[rr][:, w0:w0 + Gg, :])
                        hg = gp.tile([P, G * t_r, TW], BF16, tag=f"hg{rr}")
                        dma_gather_raw(
                            nc.gpsimd, hg[:, :Gg * t_r, :],
                            tfull[rr * rng_rows:(rr + 1) * rng_rows, 0:TW],
                            it[:, :Gg, :].rearrange("p g k -> p (g k)"),
                            nidx, TW, elem_step=P)
                        hp_g.append(hg)
                        it2 = sb.tile([P, G, K], I16, tag=f"iB{rr}")
                        nc.sync.dma_start(out=it2[:, :Gg, :],
                                          in_=iB[rr][:, w0:w0 + Gg, :])
                        ag = gp.tile([P, G * t_r, NH], F32, tag=f"ag{rr}")
                        dma_gather_raw(
                            nc.gpsimd, ag[:, :Gg * t_r, :],
                            ald[:, 0:NH] if layer == 1 else ald[:, 0:1],
                            it2[:, :Gg, :].rearrange("p g k -> p (g k)"),
                            nidx, NH, elem_step=64)
                        ald_g.append(ag)
                    dre = sb.tile([P, G, nrange * t_r], F32, tag="dre")
                    nc.sync.dma_start(out=dre[:, :Gg, :],
                                      in_=dstrel[:, w0:w0 + Gg, :])

                    # ee = exp(lrelu(al_s + al_d)) for the whole group
                    zz = gp.tile([P, nrange, G * t_r, NH], F32, tag="zz")
                    for rr in range(nrange):
                        nc.vector.tensor_tensor(
                            out=zz[:, rr, :Gg * t_r, :],
                            in0=hp_g[rr][:, :Gg * t_r, DH:DH + NH],
                            in1=ald_g[rr][:, :Gg * t_r, :],
                            op=mybir.AluOpType.add)
                    zzf = zz[:].rearrange("p r t h -> p (r t h)")
                    nc.scalar.activation(out=zzf, in_=zzf,
                                         func=mybir.ActivationFunctionType.Lrelu,
                                         alpha=SLOPE)
                    nc.scalar.activation(out=zzf, in_=zzf,
                                         func=mybir.ActivationFunctionType.Exp)
                    # messages in place: hp *= ee ; al_s cols := ee
                    for rr in range(nrange):
                        nc.vector.tensor_tensor(
                            out=hp_g[rr][:, :Gg * t_r, 0:DH].rearrange(
                                "p t (h c) -> p t h c", h=NH),
                            in0=hp_g[rr][:, :Gg * t_r, 0:DH].rearrange(
                                "p t (h c) -> p t h c", h=NH),
                            in1=zz[:, rr, :Gg * t_r, :, None].to_broadcast(
                                [P, Gg * t_r, NH, CH]),
                            op=mybir.AluOpType.mult)
                        nc.vector.tensor_copy(
                            out=hp_g[rr][:, :Gg * t_r, DH:DH + NH],
                            in_=zz[:, rr, :Gg * t_r, :])

                    for wl in range(Gg):
                        w = w0 + wl
                        rows = P if w < nwin - 1 else last_rows
                        sel = sb.tile([P, nrange * t_r, P], BF16, tag="sel")
                        nc.vector.tensor_tensor(
                            out=sel[:],
                            in0=iota_f[:].rearrange("p (t q) -> p t q", q=P),
                            in1=dre[:, wl, :, None].to_broadcast(
                                [P, nrange * t_r, P]),
                            op=mybir.AluOpType.is_equal)
                        acc = ps.tile([P, DH + NH], F32, tag="acc")
                        nmm = nrange * t_r
                        i = 0
                        for rr in range(nrange):
                            for t in range(t_r):
                                nc.tensor.matmul(
                                    acc[:],
                                    lhsT=sel[:, rr * t_r + t, :],
                                    rhs=hp_g[rr][:, wl * t_r + t, 0:DH + NH],
                                    start=(i == 0), stop=(i == nmm - 1))
                                i += 1
                        rec = sb.tile([P, NH], F32, tag="rec")
                        nc.vector.reciprocal(out=rec[:], in_=acc[:, DH:DH + NH])
                        h = sb.tile([P, DH], F32, tag="h")
                        nc.vector.tensor_tensor(
                            out=h[:].rearrange("p (h c) -> p h c", h=NH),
                            in0=acc[:, 0:DH].rearrange("p (h c) -> p h c", h=NH),
                            in1=rec[:, :, None].to_broadcast([P, NH, CH]),
                            op=mybir.AluOpType.mult)
                        nc.vector.tensor_tensor(
                            out=h[:], in0=h[:], in1=(b1s if layer == 1 else b2s)[:],
                            op=mybir.AluOpType.add)
                        if layer == 1:
                            # elu -> h ; then hp2 table rows
                            t1 = sb.tile([P, DH], F32, tag="elu1")
                            nc.vector.tensor_scalar(
                                out=t1[:], in0=h[:], scalar1=0.0, scalar2=-1.0,
                                op0=mybir.AluOpType.max,
                                op1=mybir.AluOpType.add)
                            t2 = sb.tile([P, DH], F32, tag="elu2")
                            nc.vector.tensor_scalar_min(out=t2[:], in0=h[:],
                                                        scalar1=0.0)
                            nc.scalar.activation(
                                out=t2[:], in_=t2[:],
                                func=mybir.ActivationFunctionType.Exp)
                            nc.vector.tensor_tensor(out=h[:], in0=t1[:],
                                                    in1=t2[:],
                                                    op=mybir.AluOpType.add)
                            hTp = ps1.tile([D1, P], F32, tag="hTp")
                            nc.tensor.transpose(out=hTp[:], in_=h[:],
                                                identity=ident[:])
                            hTb = sb.tile([D1, P], BF16, tag="hTb")
                            nc.vector.tensor_copy(out=hTb[:], in_=hTp[:])
                            p2 = ps1.tile([P, T2W], F32, tag="p2")
                            nc.tensor.matmul(p2[:], lhsT=hTb[:], rhs=w2s[:],
                                             start=True, stop=True)
                            t2row = sb.tile([P, T2W], BF16, tag="t2row")
                            nc.vector.tensor_copy(out=t2row[:], in_=p2[:])
                            a2row = sb.tile([P, 1], F32, tag="a2row")
                            nc.scalar.copy(out=a2row[:], in_=p2[:, D2 + 1:D2 + 2])
                            nc.sync.dma_start(
                                out=t2loc[w * P:w * P + rows, 0:T2W],
                                in_=t2row[:rows, :])
                            nc.sync.dma_start(
                                out=ald2[w * P:w * P + rows, 0:1],
                                in_=a2row[:rows, :])
                        else:
                            # log_softmax rows -> out
                            mx = sb.tile([P, 1], F32, tag="mx")
                            nc.vector.tensor_reduce(
                                out=mx[:], in_=h[:], axis=mybir.AxisListType.X,
                                op=mybir.AluOpType.max)
                            tt = sb.tile([P, D2], F32, tag="tt")
                            nc.vector.tensor_scalar(
                                out=tt[:], in0=h[:], scalar1=mx[:, 0:1],
                                scalar2=None, op0=mybir.AluOpType.subtract)
                            ex = sb.tile([P, D2], F32, tag="ex")
                            s = sb.tile([P, 1], F32, tag="s")
                            nc.scalar.activation(
                                out=ex[:], in_=tt[:],
                                func=mybir.ActivationFunctionType.Exp,
                                accum_out=s[:, 0:1])
                            ls = sb.tile([P, 1], F32, tag="ls")
                            nc.scalar.activation(
                                out=ls[:], in_=s[:],
                                func=mybir.ActivationFunctionType.Ln)
                            res = sb.tile([P, D2], F32, tag="res")
                            nc.vector.tensor_scalar(
                                out=res[:], in0=tt[:], scalar1=ls[:, 0:1],
                                scalar2=None, op0=mybir.AluOpType.subtract)
                            nc.sync.dma_start(out=out[w * P:w * P + rows, :],
                                              in_=res[:rows, :])

            edge_phase(1)
            nc.gpsimd.collective_compute(
                "AllGather", mybir.AluOpType.bypass,
                replica_groups=[list(range(ncores))],
                ins=[t2loc[:].opt()], outs=[t2full[:].opt()])
            edge_phase(2)

    nc.compile()
    return nc


# ------------------------------------------------------------------ runner
class SpmdRunner:
    def __init__(self, nc, n_cores):
        import jax
        from jax.sharding import Mesh, PartitionSpec
        from jax.experimental.shard_map import shard_map
        from concourse.bass2jax import (_bass_exec_p, partition_id_tensor,
                                        install_neuronx_cc_hook)
        install_neuronx_cc_hook()
        self.jax = jax
        self.n_cores = n_cores
        pname = nc.partition_id_tensor.name if nc.partition_id_tensor else None
        in_names, out_names, out_avals, zero_outs = [], [], [], []
        for alloc in nc.m.functions[0].allocations:
            if not isinstance(alloc, mybir.MemoryLocationSet):
                continue
            name = alloc.memorylocations[0].name
            if alloc.kind == "ExternalInput":
                if name != pname:
                    in_names.append(name)
            elif alloc.kind == "ExternalOutput":
                out_names.append(name)
                out_avals.append(jax.core.ShapedArray(
                    tuple(alloc.tensor_shape), mybir.dt.np(alloc.dtype)))
                zero_outs.append(np.zeros(tuple(alloc.tensor_shape),
                                          mybir.dt.np(alloc.dtype)))
        self.in_names, self.out_names = in_names, out_names
        self.out_avals, self.zero_outs = out_avals, zero_outs
        self.n_params = len(in_names)
        all_in = in_names + out_names + ([pname] if pname else [])

        def _body(*args):
            operands = list(args)
            if pname is not None:
                operands.append(partition_id_tensor())
            return tuple(_bass_exec_p.bind(
                *operands, out_avals=tuple(out_avals), in_names=tuple(all_in),
                out_names=tuple(out_names), lowering_input_output_aliases=(),
                sim_require_finite=True, sim_require_nnan=True, nc=nc))

        donate = tuple(range(self.n_params, self.n_params + len(out_avals)))
        devices = jax.devices()[:n_cores]
        self.mesh = Mesh(np.asarray(devices), ("core",))
        self.pspec = PartitionSpec("core")
        in_specs = (self.pspec,) * (self.n_params + len(out_avals))
        out_specs = (self.pspec,) * len(out_avals)
        self.sharded = jax.jit(
            shard_map(_body, mesh=self.mesh, in_specs=in_specs,
                      out_specs=out_specs, check_rep=False),
            donate_argnums=donate, keep_unused=True)

    def run(self, in_maps, reps=1):
        import time
        from jax.sharding import NamedSharding
        jax = self.jax
        sh = NamedSharding(self.mesh, self.pspec)
        per_core = [[np.asarray(m[name]) for name in self.in_names]
                    for m in in_maps]
        concat = [np.concatenate([per_core[c][i] for c in range(self.n_cores)],
                                 axis=0) for i in range(self.n_params)]
        dev_in = [jax.device_put(a, sh) for a in concat]
        best = float("inf")
        out_arrs = None
        for _ in range(reps):
            dz = [jax.device_put(
                np.zeros((self.n_cores * z.shape[0], *z.shape[1:]), z.dtype), sh)
                for z in self.zero_outs]
            for a in dz:
                a.block_until_ready()
            t0 = time.perf_counter_ns()
            out_arrs = self.sharded(*dev_in, *dz)
            for a in out_arrs:
                a.block_until_ready()
            best = min(best, time.perf_counter_ns() - t0)
        results = [
            {name: np.asarray(out_arrs[i]).reshape(
                self.n_cores, *self.out_avals[i].shape)[c]
             for i, name in enumerate(self.out_names)}
            for c in range(self.n_cores)]
        return results, best


# ----------------------------------------------------------------- kernel()
def make_cfg(N, E, F_IN, H1, C1, C2, ncores):
    nloc = N // ncores
    return dict(N=N, E=E, F_IN=F_IN, H1=H1, C1=C1, C2=C2, ncores=ncores,
                nloc=nloc, nwin=math.ceil(nloc / P), nrange=4)


DEFAULT_CFG = make_cfg(N=100000, E=1600000, F_IN=512, H1=8, C1=8, C2=16,
                       ncores=8)


def fold_weights(W1, a1_src, a1_dst, W2, a2_src, a2_dst, cfg):
    H1, C1 = cfg["H1"], cfg["C1"]
    W1r = W1.reshape(cfg["F_IN"], H1, C1)
    w1s = np.einsum("khc,hc->kh", W1r, a1_src)
    w1d = np.einsum("khc,hc->kh", W1r, a1_dst)
    W1e = np.concatenate([W1, w1s, w1d], axis=1).astype(bf16)
    w2s = W2 @ a2_src[0]
    w2d = W2 @ a2_dst[0]
    W2e = np.concatenate([W2, w2s[:, None], w2d[:, None]], axis=1).astype(bf16)
    return W1e, W2e


_CACHE = {}


def prepare(inputs, cfg=DEFAULT_CFG, reps=1):
    x = np.asarray(inputs["x"], np.float32)
    edge_index = np.asarray(inputs["edge_index"])
    W1 = np.asarray(inputs["W1"], np.float32)
    W2 = np.asarray(inputs["W2"], np.float32)
    b1 = np.asarray(inputs["b1"], np.float32)
    b2 = np.asarray(inputs["b2"], np.float32)
    a1s = np.asarray(inputs["a1_src"], np.float32)
    a1d = np.asarray(inputs["a1_dst"], np.float32)
    a2s = np.asarray(inputs["a2_src"], np.float32)
    a2d = np.asarray(inputs["a2_dst"], np.float32)

    per_core_idx, t_r = preprocess(edge_index, cfg)
    key = (cfg["N"], t_r)
    if key not in _CACHE:
        nc = build_nc(cfg, t_r)
        _CACHE[key] = (nc, SpmdRunner(nc, cfg["ncores"]))
    nc, runner = _CACHE[key]

    W1e, W2e = fold_weights(W1, a1s, a1d, W2, a2s, a2d, cfg)
    b1rep = np.tile(b1[None, :], (P, 1)).astype(np.float32)
    b2rep = np.tile(b2[None, :], (P, 1)).astype(np.float32)
    nloc = cfg["nloc"]
    in_maps = []
    for c in range(cfg["ncores"]):
        m = dict(per_core_idx[c])
        m["xT"] = np.ascontiguousarray(
            x[c * nloc:(c + 1) * nloc, :].T).astype(bf16)
        m["W1e"], m["W2e"] = W1e, W2e
        m["b1r"], m["b2r"] = b1rep, b2rep
        in_maps.append(m)
    return runner, in_maps


def kernel_timed(inputs, reps=1):
    cfg = DEFAULT_CFG
    runner, in_maps = prepare(inputs, cfg, reps)
    results, best_ns = runner.run(in_maps, reps=reps)
    out = np.concatenate([results[c]["out"] for c in range(cfg["ncores"])],
                         axis=0)
    return out, best_ns


def kernel(**inputs):
    out, _ = kernel_timed(inputs, reps=1)
    return out



# revision 16
# speedup vs baseline: 1.4619x; 1.4619x over previous
"""2-layer GAT (gnn_message_passing) on 8 Trainium2 NeuronCores.

Strategy (per sharding hint): nodes are partitioned contiguously across the 8
cores (12500 each). Edges (incl. self-loops) are sharded by destination core
and bucketed by (destination window of 128 nodes, source class k=0..3), with
per-bucket tile counts (max over cores, so the program is SPMD-uniform) and a
group-contiguous slot layout so each DMA gather is one contiguous run.

Layer 1 buckets by source range (src//25000) so gather indices fit int16;
layer 2 buckets by source parity (src%4) against a 4-node-packed table so the
AllGather moves 6.4MB instead of 25.6MB. Per layer: a dense phase computes
hp = x @ W (attention logit halves folded into extra weight columns), the
row table is AllGathered (compact, Shared output) and locally re-strided to
the 256B row pitch the DMA gather needs. The edge phase gathers rows by edge
source, gathers al_dst by destination, forms ee = exp(leaky_relu(al_s+al_d))
(leaky_relu on DVE so the activation table stays on Exp), multiplies messages,
and scatter-adds per destination window with one-hot selection matmuls on the
PE (which also accumulate softmax denominators). log_softmax runs as one bulk
pass at the end (single Exp + single Ln table load).
"""
import math
import numpy as np
import ml_dtypes

import concourse.bacc as bacc
import concourse.mybir as mybir
import concourse.tile as tile
from concourse import ap_utils

bf16 = ml_dtypes.bfloat16
F32 = mybir.dt.float32
BF16 = mybir.dt.bfloat16
I16 = mybir.dt.int16
I32 = mybir.dt.int32

P = 128
TMAXK = 30      # tiles per (group, k) gather call: 30*128 = 3840 idxs
TMAXT = 96      # total tiles per group (SBUF budget for hg/sel)
GA = 7          # phase-A windows per group
SLOPE = 0.2


# ---------------------------------------------------------------- dma_gather
def dma_gather_raw(eng, out_ap, in_ap, idxs_ap, num_idxs, elem_size,
                   elem_step=None, queue_num=0):
    """BassGpSimd.dma_gather (DRAM src, non-transpose) minus the
    elem_size%256B assert (transpose-only restriction) and with
    single_packet=False (large single packets wedge the SDMA)."""
    assert idxs_ap.dtype == mybir.dt.int16
    assert in_ap.dtype == out_ap.dtype
    elem_size_bytes = elem_size * mybir.dt.size(in_ap.dtype)
    assert elem_size_bytes > 0
    if elem_step is None:
        elem_step = elem_size
    assert ap_utils.ap_is_contiguous(in_ap.ap[1:])
    assert ap_utils.ap_is_contiguous(out_ap.ap[1:])
    assert ap_utils.ap_is_contiguous(idxs_ap.ap[1:])
    assert in_ap.ap[0][0] == elem_step
    assert in_ap.ap[-1][1] == elem_size
    assert out_ap.ap[-1][1] == elem_size
    assert num_idxs <= TMAXK * P + 256
    stride_bytes = elem_step * mybir.dt.size(in_ap.dtype)
    assert stride_bytes % 256 == 0 and stride_bytes // 256 < 256
    _in_ap = eng.lower_ap_dma(in_ap, for_custom_bir_dma=True)
    _idxs_ap = eng.lower_ap(idxs_ap)
    _out_ap = eng.lower_ap(out_ap)
    return eng.add_instruction(
        mybir.InstDMAGatherAnt(
            name=eng.bass.get_next_instruction_name(),
            ins=[*_in_ap, _idxs_ap, eng.lower_val_access(eng.to_reg(num_idxs))],
            outs=[_out_ap],
            transpose=False,
            num_idxs=num_idxs,
            elem_size=elem_size,
            stride_bytes_256=stride_bytes // 256,
            gen_mode=0,
            single_packet=False,
            queue_num=queue_num,
            sbuf_tokens_per_rank=0,
            sbuf_free_dim_per_rank=0,
            sbuf_free_dim_pad_per_rank=0,
            sbuf_byte_offset=0,
        )
    )


# ------------------------------------------------------------- host preproc
def _wrap_flat(a):
    """[S] int -> [128, S//16] int16 dma_gather idx layout (idx j at lane
    j%16 col j//16, replicated to 8 lane groups)."""
    w = a.reshape(-1, 16).T
    return np.ascontiguousarray(np.tile(w, (8, 1)).astype(np.int16))


class Meta:
    """Static (core-uniform) slot structure for one bucketing scheme."""

    def __init__(self, tiles):
        nwin = tiles.shape[0]
        groups = []
        w = 0
        while w < nwin:
            ws = []
            per_k = np.zeros(4, np.int64)
            tot = 0
            while w < nwin:
                t = tiles[w]
                if ws and (np.any(per_k + t > TMAXK) or tot + t.sum() > TMAXT):
                    break
                ws.append(w)
                per_k += t
                tot += int(t.sum())
                w += 1
            groups.append(ws)
        self.bucket_tile0 = np.zeros((nwin, 4), np.int64)
        self.groups = []
        tidx = 0
        for ws in groups:
            g = {"windows": ws, "tile0": tidx, "k_off": [], "k_tiles": []}
            for k in range(4):
                g["k_off"].append(tidx - g["tile0"])
                n = 0
                for wi in ws:
                    self.bucket_tile0[wi, k] = tidx
                    tidx += int(tiles[wi, k])
                    n += int(tiles[wi, k])
                g["k_tiles"].append(n)
            g["T"] = tidx - g["tile0"]
            self.groups.append(g)
        self.tiles = tiles
        self.n_tiles = tidx
        self.S = tidx * P

    def window_tiles(self, w):
        """Global tile indices feeding window w, in (k, tile) order."""
        out = []
        for k in range(4):
            b0 = int(self.bucket_tile0[w, k])
            out.extend(range(b0, b0 + int(self.tiles[w, k])))
        return out


def _scheme_arrays(cfg, src, dst, k, sidx, tiles):
    """Per-core flat slot arrays for one scheme."""
    N, ncores, nloc, nwin = cfg["N"], cfg["ncores"], cfg["nloc"], cfg["nwin"]
    meta = Meta(tiles)
    core = dst // nloc
    dst_loc = dst - core * nloc
    w = dst_loc // P
    dst_rel = dst_loc - w * P
    key = (core * nwin + w) * 4 + k
    counts = np.bincount(key, minlength=ncores * nwin * 4)
    starts = np.zeros(ncores * nwin * 4 + 1, np.int64)
    np.cumsum(counts, out=starts[1:])
    order = np.argsort(key, kind="stable")
    ks = key[order]
    pos = np.arange(len(ks)) - starts[ks]
    slot0 = meta.bucket_tile0[w, k] * P          # per edge (core-uniform)
    slot = np.empty(len(ks), np.int64)
    slot[order] = (slot0[order] + pos)
    S = meta.S
    per_core = []
    for c in range(ncores):
        m = core == c
        sidx_f = np.zeros(S, np.int16)
        dloc_f = np.zeros(S, np.int16)
        drel_f = np.full(S, -1.0, np.float32)
        sidx_f[slot[m]] = sidx[m].astype(np.int16)
        dloc_f[slot[m]] = dst_loc[m].astype(np.int16)
        drel_f[slot[m]] = dst_rel[m].astype(np.float32)
        dre_dev = drel_f.reshape(S // P, P).T.astype(bf16)   # [P, n_tiles]
        per_core.append({
            "iA": _wrap_flat(sidx_f),
            "iB": _wrap_flat(dloc_f),
            # doubled innermost (value at cols 2t, 2t+1) so the one-hot
            # compare keeps the DVE 2x packed fast path
            "dre": np.ascontiguousarray(np.repeat(dre_dev, 2, axis=1)),
        })
    return meta, per_core


def preprocess(edge_index, cfg):
    N, ncores, nloc, nwin = cfg["N"], cfg["ncores"], cfg["nloc"], cfg["nwin"]
    NR = N // 4
    loops = np.arange(N, dtype=np.int64)
    src = np.concatenate([edge_index[0].astype(np.int64), loops])
    dst = np.concatenate([edge_index[1].astype(np.int64), loops])
    core = dst // nloc
    w = (dst - core * nloc) // P

    def tiles_for(k):
        key = (core * nwin + w) * 4 + k
        cnt = np.bincount(key, minlength=ncores * nwin * 4)
        cnt = cnt.reshape(ncores, nwin, 4).max(axis=0)
        return np.maximum((cnt + P - 1) // P, 0).astype(np.int64)

    k1 = src // NR
    k2 = src % 4
    tiles1 = tiles_for(k1)
    tiles2 = tiles_for(k2)
    meta1, pc1 = _scheme_arrays(cfg, src, dst, k1, src - k1 * NR, tiles1)
    meta2, pc2 = _scheme_arrays(cfg, src, dst, k2, src // 4, tiles2)
    per_core = []
    for c in range(ncores):
        m = {"iA1": pc1[c]["iA"], "iB1": pc1[c]["iB"], "dre1": pc1[c]["dre"],
             "iA2": pc2[c]["iA"], "iB2": pc2[c]["iB"], "dre2": pc2[c]["dre"]}
        per_core.append(m)
    return meta1, meta2, per_core


# ------------------------------------------------------------- device build
def build_nc(cfg, meta1, meta2):
    N, ncores, nloc, nwin = cfg["N"], cfg["ncores"], cfg["nloc"], cfg["nwin"]
    F_IN, H1, C2 = cfg["F_IN"], cfg["H1"], cfg["C2"]
    D1 = 64
    A1 = D1 + 2 * H1        # 80: [hp | al_s | al_d]
    T1W = D1 + H1           # 72
    D2 = C2                 # 16
    T2W = D2 + 2            # 18
    NR = N // 4
    kchunks = F_IN // P
    TMG = max(max(g["T"] for g in meta1.groups),
              max(g["T"] for g in meta2.groups))
    GWMAX = max(max(len(g["windows"]) for g in meta1.groups),
                max(len(g["windows"]) for g in meta2.groups), GA)

    nc = bacc.Bacc("TRN2", target_bir_lowering=False, num_devices=ncores)
    xT = nc.dram_tensor("xT", [F_IN, nloc], BF16, kind="ExternalInput")
    W1e = nc.dram_tensor("W1e", [F_IN, A1], BF16, kind="ExternalInput")
    W2e = nc.dram_tensor("W2e", [D1, T2W], BF16, kind="ExternalInput")
    b1r = nc.dram_tensor("b1r", [P, D1], F32, kind="ExternalInput")
    b2r = nc.dram_tensor("b2r", [P, D2], F32, kind="ExternalInput")
    iA1 = nc.dram_tensor("iA1", [P, meta1.S // 16], I16, kind="ExternalInput")
    iB1 = nc.dram_tensor("iB1", [P, meta1.S // 16], I16, kind="ExternalInput")
    dre1 = nc.dram_tensor("dre1", [P, 2 * meta1.n_tiles], BF16,
                          kind="ExternalInput")
    iA2 = nc.dram_tensor("iA2", [P, meta2.S // 16], I16, kind="ExternalInput")
    iB2 = nc.dram_tensor("iB2", [P, meta2.S // 16], I16, kind="ExternalInput")
    dre2 = nc.dram_tensor("dre2", [P, 2 * meta2.n_tiles], BF16,
                          kind="ExternalInput")
    out = nc.dram_tensor("out", [nloc, D2], F32, kind="ExternalOutput")

    with tile.TileContext(nc) as tc:
        with (
            tc.tile_pool(name="const", bufs=1) as cpool,
            tc.tile_pool(name="sbuf", bufs=2) as sb,
            tc.tile_pool(name="gat", bufs=2) as gp,
            tc.tile_pool(name="selp", bufs=2) as sp,
            tc.tile_pool(name="psum", bufs=2, space="PSUM") as ps,
            tc.tile_pool(name="psum1", bufs=2, space="PSUM") as ps1,
            tc.tile_pool(name="dram", bufs=1, space="DRAM") as dr,
        ):
            t1loc = dr.tile([nloc, T1W], BF16)
            t1cf = dr.tile([N, T1W], BF16, addr_space="Shared")
            t1g = dr.tile([N, P], BF16)
            ald1 = dr.tile([nloc, P], BF16)
            t2loc = dr.tile([nloc // 4, P], BF16)
            t2g = dr.tile([N // 4, P], BF16, addr_space="Shared")
            ald2 = dr.tile([nloc, P], BF16)
            outr = dr.tile([nloc, D2], F32)

            # ---- static constants
            w1s = cpool.tile([P, kchunks, A1], BF16)
            nc.sync.dma_start(out=w1s[:], in_=W1e[:].rearrange(
                "(c p) a -> p c a", p=P))
            w2s = cpool.tile([D1, T2W], BF16)
            nc.sync.dma_start(out=w2s[:], in_=W2e[:])
            b1s = cpool.tile([P, D1], F32)
            nc.sync.dma_start(out=b1s[:], in_=b1r[:])
            b2s = cpool.tile([P, D2], F32)
            nc.sync.dma_start(out=b2s[:], in_=b2r[:])
            iota_i = cpool.tile([P, P], I32)
            nc.gpsimd.iota(iota_i[:], pattern=[[1, P]],
                           base=0, channel_multiplier=0)
            iota_f = cpool.tile([P, P], F32)
            nc.vector.tensor_copy(out=iota_f[:], in_=iota_i[:])
            iota_b = cpool.tile([P, P], BF16)
            nc.vector.tensor_copy(out=iota_b[:], in_=iota_f[:])
            ident = cpool.tile([P, P], F32)
            from concourse.masks import make_identity
            make_identity(nc, ident[:])

            # ---- phase A: layer-1 node table
            for ga in range(math.ceil(nwin / GA)):
                w0 = ga * GA
                gw = min(GA, nwin - w0)
                r0 = w0 * P
                rows_g = min(nloc, (w0 + gw) * P) - r0
                xa = sb.tile([P, kchunks, GA * P], BF16, tag="xa")
                nc.sync.dma_start(
                    out=xa[:, :, :rows_g],
                    in_=xT[:, r0:r0 + rows_g].rearrange("(c p) r -> p c r",
                                                        p=P))
                t1rows = sb.tile([P, GA, A1], BF16, tag="t1rows")
                for wl in range(gw):
                    pA = ps.tile([P, A1], F32, tag="pA")
                    for c in range(kchunks):
                        nc.tensor.matmul(pA[:],
                                         lhsT=xa[:, c, wl * P:(wl + 1) * P],
                                         rhs=w1s[:, c, :],
                                         start=(c == 0),
                                         stop=(c == kchunks - 1))
                    nc.scalar.copy(out=t1rows[:, wl, :], in_=pA[:])
                n_full = rows_g // P
                if n_full:
                    nc.sync.dma_start(
                        out=t1loc[r0:r0 + n_full * P, :].rearrange(
                            "(g p) c -> p g c", p=P),
                        in_=t1rows[:, :n_full, 0:T1W])
                    nc.sync.dma_start(
                        out=ald1[r0:r0 + n_full * P, 0:H1].rearrange(
                            "(g p) c -> p g c", p=P),
                        in_=t1rows[:, :n_full, T1W:A1])
                tail = rows_g - n_full * P
                if tail:
                    nc.sync.dma_start(
                        out=t1loc[r0 + n_full * P:r0 + rows_g, :],
                        in_=t1rows[:tail, n_full, 0:T1W])
                    nc.sync.dma_start(
                        out=ald1[r0 + n_full * P:r0 + rows_g, 0:H1],
                        in_=t1rows[:tail, n_full, T1W:A1])

            # ---- allgather T1 (compact) + local re-stride to 256B pitch
            nc.gpsimd.collective_compute(
                "AllGather", mybir.AluOpType.bypass,
                replica_groups=[list(range(ncores))],
                ins=[t1loc[:].opt()], outs=[t1cf[:].opt()])
            for rr in range(4):
                nc.sync.dma_start(out=t1g[rr * NR:(rr + 1) * NR, 0:T1W],
                                  in_=t1cf[rr * NR:(rr + 1) * NR, :])

            # ---- edge phases
            def edge_phase(layer):
                meta = meta1 if layer == 1 else meta2
                iA, iB, dre = (iA1, iB1, dre1) if layer == 1 else \
                              (iA2, iB2, dre2)
                ald = ald1 if layer == 1 else ald2
                TW = T1W if layer == 1 else T2W
                DH = D1 if layer == 1 else D2
                NH = H1 if layer == 1 else 1
                CH = DH // NH
                AW = DH + NH                       # scatter payload width
                for grp in meta.groups:
                    T = grp["T"]
                    t0 = grp["tile0"]
                    hg = gp.tile([P, TMG, TW], BF16, tag="hg")
                    agt = gp.tile([P, TMG, NH], BF16, tag="agt")
                    ia = sb.tile([P, TMG * 8], I16, tag="ia")
                    nc.sync.dma_start(out=ia[:, :T * 8],
                                      in_=iA[:, t0 * 8:(t0 + T) * 8])
                    ib = sb.tile([P, TMG * 8], I16, tag="ib")
                    nc.sync.dma_start(out=ib[:, :T * 8],
                                      in_=iB[:, t0 * 8:(t0 + T) * 8])
                    dre_t = sb.tile([P, TMG, 2], BF16, tag="dre")
                    nc.sync.dma_start(
                        out=dre_t[:, :T, :],
                        in_=dre[:, 2 * t0:2 * (t0 + T)].rearrange(
                            "p (t j) -> p t j", j=2))
                    for k in range(4):
                        off, ntk = grp["k_off"][k], grp["k_tiles"][k]
                        if ntk == 0:
                            continue
                        nidx = ntk * P
                        if layer == 1:
                            src_ap = t1g[k * NR:(k + 1) * NR, 0:TW]
                        else:
                            src_ap = t2g[:, 32 * k:32 * k + TW]
                        dma_gather_raw(
                            nc.gpsimd, hg[:, off:off + ntk, :], src_ap,
                            ia[:, off * 8:(off + ntk) * 8], nidx, TW,
                            elem_step=P)
                        dma_gather_raw(
                            nc.gpsimd, agt[:, off:off + ntk, :],
                            ald[:, 0:NH],
                            ib[:, off * 8:(off + ntk) * 8], nidx, NH,
                            elem_step=P)
                    # ee = exp(lrelu(al_s + al_d)); lrelu on DVE
                    zz = gp.tile([P, TMG, NH], BF16, tag="zz")
                    nc.vector.tensor_tensor(out=zz[:, :T, :],
                                            in0=hg[:, :T, DH:DH + NH],
                                            in1=agt[:, :T, :],
                                            op=mybir.AluOpType.add)
                    zz2 = gp.tile([P, TMG, NH], BF16, tag="zz2")
                    nc.vector.tensor_scalar(out=zz2[:, :T, :],
                                            in0=zz[:, :T, :],
                                            scalar1=SLOPE, scalar2=None,
                                            op0=mybir.AluOpType.mult)
                    nc.vector.tensor_tensor(out=zz[:, :T, :],
                                            in0=zz[:, :T, :],
                                            in1=zz2[:, :T, :],
                                            op=mybir.AluOpType.max)
                    nc.scalar.activation(
                        out=zz[:, :T, :], in_=zz[:, :T, :],
                        func=mybir.ActivationFunctionType.Exp)
                    # messages: hp *= ee ; al_s cols := ee (denominators).
                    # ee is duplicated x2 innermost so the mult keeps the
                    # DVE 2x packed fast path.
                    zzd = gp.tile([P, TMG, NH, 2], BF16, tag="zzd")
                    nc.vector.tensor_copy(
                        out=zzd[:, :T, :, :],
                        in_=zz[:, :T, :, None].to_broadcast([P, T, NH, 2]))
                    nc.vector.tensor_tensor(
                        out=hg[:, :T, 0:DH].rearrange(
                            "p t (h c b) -> p t h c b", h=NH, b=2),
                        in0=hg[:, :T, 0:DH].rearrange(
                            "p t (h c b) -> p t h c b", h=NH, b=2),
                        in1=zzd[:, :T, :, None, :].to_broadcast(
                            [P, T, NH, CH // 2, 2]),
                        op=mybir.AluOpType.mult)
                    nc.vector.tensor_copy(out=hg[:, :T, DH:DH + NH],
                                          in_=zz[:, :T, :])
                    # one-hot selection for the whole group
                    sel = sp.tile([P, TMG, P], BF16, tag="sel")
                    nc.vector.tensor_tensor(
                        out=sel[:, :T, :].rearrange(
                            "p t (a b) -> p t a b", b=2),
                        in0=iota_b[:].rearrange(
                            "p (a b) -> p a b", b=2)[:, None, :, :]
                        .to_broadcast([P, T, P // 2, 2]),
                        in1=dre_t[:, :T, None, :].to_broadcast(
                            [P, T, P // 2, 2]),
                        op=mybir.AluOpType.is_equal)
                    # scatter per window
                    GW = len(grp["windows"])
                    hfin = sb.tile([P, GWMAX, AW], F32, tag="hfin")
                    for wi, w in enumerate(grp["windows"]):
                        lts = [t - t0 for t in meta.window_tiles(w)]
                        acc = ps.tile([P, AW], F32, tag="acc")
                        for i, lt in enumerate(lts):
                            nc.tensor.matmul(acc[:],
                                             lhsT=sel[:, lt, :],
                                             rhs=hg[:, lt, 0:AW],
                                             start=(i == 0),
                                             stop=(i == len(lts) - 1))
                        nc.scalar.copy(out=hfin[:, wi, :], in_=acc[:])
                    # normalize + bias (batched over the group's windows)
                    nc.vector.tensor_scalar(out=hfin[:, :GW, DH:DH + NH],
                                            in0=hfin[:, :GW, DH:DH + NH],
                                            scalar1=1e-20, scalar2=None,
                                            op0=mybir.AluOpType.max)
                    rec = sb.tile([P, GWMAX, NH], F32, tag="rec")
                    nc.vector.reciprocal(out=rec[:, :GW, :],
                                         in_=hfin[:, :GW, DH:DH + NH])
                    nc.vector.tensor_tensor(
                        out=hfin[:, :GW, 0:DH].rearrange(
                            "p g (h c) -> p g h c", h=NH),
                        in0=hfin[:, :GW, 0:DH].rearrange(
                            "p g (h c) -> p g h c", h=NH),
                        in1=rec[:, :GW, :, None].to_broadcast(
                            [P, GW, NH, CH]),
                        op=mybir.AluOpType.mult)
                    nc.vector.tensor_tensor(
                        out=hfin[:, :GW, 0:DH],
                        in0=hfin[:, :GW, 0:DH],
                        in1=(b1s if layer == 1 else b2s)[:, None, :]
                        .to_broadcast([P, GW, DH]),
                        op=mybir.AluOpType.add)
                    w0 = grp["windows"][0]
                    r0 = w0 * P
                    rows_g = min(nloc, (w0 + GW) * P) - r0
                    n_full = rows_g // P
                    tail = rows_g - n_full * P
                    if layer == 1:
                        # elu(h) then layer-2 table rows
                        h = hfin[:, :GW, 0:DH]
                        e1 = sb.tile([P, GWMAX, DH], F32, tag="e1")
                        nc.vector.tensor_scalar(out=e1[:, :GW, :], in0=h,
                                                scalar1=0.0, scalar2=-1.0,
                                                op0=mybir.AluOpType.max,
                                                op1=mybir.AluOpType.add)
                        nc.vector.tensor_scalar_min(out=h, in0=h, scalar1=0.0)
                        nc.scalar.activation(
                            out=h, in_=h,
                            func=mybir.ActivationFunctionType.Exp)
                        nc.vector.tensor_tensor(out=h, in0=h,
                                                in1=e1[:, :GW, :],
                                                op=mybir.AluOpType.add)
                        t2rows = sb.tile([P, GWMAX, T2W], BF16, tag="t2rows")
                        for wi in range(GW):
                            hTp = ps1.tile([D1, P], F32, tag="hTp")
                            nc.tensor.transpose(out=hTp[:],
                                                in_=hfin[:, wi, 0:D1],
                                                identity=ident[:])
                            hTb = sb.tile([D1, P], BF16, tag="hTb")
                            nc.scalar.copy(out=hTb[:], in_=hTp[:])
                            p2 = ps1.tile([P, T2W], F32, tag="p2")
                            nc.tensor.matmul(p2[:], lhsT=hTb[:], rhs=w2s[:],
                                             start=True, stop=True)
                            nc.scalar.copy(out=t2rows[:, wi, :], in_=p2[:])
                        # packed (4 nodes / 256B row) table store + ald2
                        if n_full:
                            nc.sync.dma_start(
                                out=t2loc[w0 * 32:(w0 + n_full) * 32, :]
                                .rearrange("(g a) (b c) -> (a b) g c",
                                           a=32, b=4)[:, :, 0:T2W],
                                in_=t2rows[:, :n_full, :])
                            nc.sync.dma_start(
                                out=ald2[r0:r0 + n_full * P, 0:1].rearrange(
                                    "(g p) c -> p g c", p=P),
                                in_=t2rows[:, :n_full, T2W - 1:T2W])
                        if tail:
                            wt = w0 + n_full
                            nc.sync.dma_start(
                                out=t2loc[wt * 32:wt * 32 + tail // 4, :]
                                .rearrange("a (b c) -> (a b) c",
                                           b=4)[:tail, 0:T2W],
                                in_=t2rows[:tail, n_full, :])
                            nc.sync.dma_start(
                                out=ald2[r0 + n_full * P:r0 + rows_g, 0:1],
                                in_=t2rows[:tail, n_full, T2W - 1:T2W])
                    else:
                        if n_full:
                            nc.sync.dma_start(
                                out=outr[r0:r0 + n_full * P, :].rearrange(
                                    "(g p) c -> p g c", p=P),
                                in_=hfin[:, :n_full, 0:D2])
                        if tail:
                            nc.sync.dma_start(
                                out=outr[r0 + n_full * P:r0 + rows_g, :],
                                in_=hfin[:tail, n_full, 0:D2])

            edge_phase(1)
            nc.gpsimd.collective_compute(
                "AllGather", mybir.AluOpType.bypass,
                replica_groups=[list(range(ncores))],
                ins=[t2loc[:].opt()], outs=[t2g[:].opt()])
            edge_phase(2)

            # ---- bulk log_softmax over all local rows
            nw_full = nloc // P
            tail = nloc - nw_full * P
            nw = nw_full + (1 if tail else 0)
            hb = sb.tile([P, nw, D2], F32, tag="hb")
            nc.sync.dma_start(
                out=hb[:, :nw_full, :],
                in_=outr[0:nw_full * P, :].rearrange("(c p) d -> p c d", p=P))
            if tail:
                nc.sync.dma_start(out=hb[:tail, nw_full, :],
                                  in_=outr[nw_full * P:nloc, :])
            mx = sb.tile([P, nw, 1], F32, tag="mx")
            nc.vector.tensor_reduce(out=mx[:], in_=hb[:],
                                    axis=mybir.AxisListType.X,
                                    op=mybir.AluOpType.max)
            nc.vector.tensor_tensor(
                out=hb[:], in0=hb[:],
                in1=mx[:, :, 0, None].to_broadcast([P, nw, D2]),
                op=mybir.AluOpType.subtract)
            ex = sb.tile([P, nw, D2], F32, tag="ex")
            nc.scalar.activation(out=ex[:], in_=hb[:],
                                 func=mybir.ActivationFunctionType.Exp)
            sm = sb.tile([P, nw, 1], F32, tag="sm")
            nc.vector.tensor_reduce(out=sm[:], in_=ex[:],
                                    axis=mybir.AxisListType.X,
                                    op=mybir.AluOpType.add)
            ls = sb.tile([P, nw, 1], F32, tag="ls")
            nc.scalar.activation(out=ls[:], in_=sm[:],
                                 func=mybir.ActivationFunctionType.Ln)
            nc.vector.tensor_tensor(
                out=hb[:], in0=hb[:],
                in1=ls[:, :, 0, None].to_broadcast([P, nw, D2]),
                op=mybir.AluOpType.subtract)
            nc.sync.dma_start(
                out=out[0:nw_full * P, :].rearrange("(c p) d -> p c d", p=P),
                in_=hb[:, :nw_full, :])
            if tail:
                nc.sync.dma_start(out=out[nw_full * P:nloc, :],
                                  in_=hb[:tail, nw_full, :])

    nc.compile()
    return nc


# ------------------------------------------------------------------ runner
class SpmdRunner:
    def __init__(self, nc, n_cores):
        import jax
        from jax.sharding import Mesh, PartitionSpec
        from jax.experimental.shard_map import shard_map
        from concourse.bass2jax import (_bass_exec_p, partition_id_tensor,
                                        install_neuronx_cc_hook)
        install_neuronx_cc_hook()
        self.jax = jax
        self.n_cores = n_cores
        pname = nc.partition_id_tensor.name if nc.partition_id_tensor else None
        in_names, out_names, out_avals, zero_outs = [], [], [], []
        for alloc in nc.m.functions[0].allocations:
            if not isinstance(alloc, mybir.MemoryLocationSet):
                continue
            name = alloc.memorylocations[0].name
            if alloc.kind == "ExternalInput":
                if name != pname:
                    in_names.append(name)
            elif alloc.kind == "ExternalOutput":
                out_names.append(name)
                out_avals.append(jax.core.ShapedArray(
                    tuple(alloc.tensor_shape), mybir.dt.np(alloc.dtype)))
                zero_outs.append(np.zeros(tuple(alloc.tensor_shape),
                                          mybir.dt.np(alloc.dtype)))
        self.in_names, self.out_names = in_names, out_names
        self.out_avals, self.zero_outs = out_avals, zero_outs
        self.n_params = len(in_names)
        all_in = in_names + out_names + ([pname] if pname else [])

        def _body(*args):
            operands = list(args)
            if pname is not None:
                operands.append(partition_id_tensor())
            return tuple(_bass_exec_p.bind(
                *operands, out_avals=tuple(out_avals), in_names=tuple(all_in),
                out_names=tuple(out_names), lowering_input_output_aliases=(),
                sim_require_finite=True, sim_require_nnan=True, nc=nc))

        donate = tuple(range(self.n_params, self.n_params + len(out_avals)))
        devices = jax.devices()[:n_cores]
        self.mesh = Mesh(np.asarray(devices), ("core",))
        self.pspec = PartitionSpec("core")
        in_specs = (self.pspec,) * (self.n_params + len(out_avals))
        out_specs = (self.pspec,) * len(out_avals)
        self.sharded = jax.jit(
            shard_map(_body, mesh=self.mesh, in_specs=in_specs,
                      out_specs=out_specs, check_rep=False),
            donate_argnums=donate, keep_unused=True)

    def run(self, in_maps, reps=1):
        import time
        from jax.sharding import NamedSharding
        jax = self.jax
        sh = NamedSharding(self.mesh, self.pspec)
        per_core = [[np.asarray(m[name]) for name in self.in_names]
                    for m in in_maps]
        concat = [np.concatenate([per_core[c][i] for c in range(self.n_cores)],
                                 axis=0) for i in range(self.n_params)]
        dev_in = [jax.device_put(a, sh) for a in concat]
        best = float("inf")
        out_arrs = None
        for _ in range(reps):
            dz = [jax.device_put(
                np.zeros((self.n_cores * z.shape[0], *z.shape[1:]), z.dtype), sh)
                for z in self.zero_outs]
            for a in dz:
                a.block_until_ready()
            t0 = time.perf_counter_ns()
            out_arrs = self.sharded(*dev_in, *dz)
            for a in out_arrs:
                a.block_until_ready()
            best = min(best, time.perf_counter_ns() - t0)
        results = [
            {name: np.asarray(out_arrs[i]).reshape(
                self.n_cores, *self.out_avals[i].shape)[c]
             for i, name in enumerate(self.out_names)}
            for c in range(self.n_cores)]
        return results, best


# ----------------------------------------------------------------- kernel()
def make_cfg(N, E, F_IN, H1, C1, C2, ncores):
    nloc = N // ncores
    return dict(N=N, E=E, F_IN=F_IN, H1=H1, C1=C1, C2=C2, ncores=ncores,
                nloc=nloc, nwin=math.ceil(nloc / P))


DEFAULT_CFG = make_cfg(N=100000, E=1600000, F_IN=512, H1=8, C1=8, C2=16,
                       ncores=8)


def fold_weights(W1, a1_src, a1_dst, W2, a2_src, a2_dst, cfg):
    H1, C1 = cfg["H1"], cfg["C1"]
    W1r = W1.reshape(cfg["F_IN"], H1, C1)
    w1s = np.einsum("khc,hc->kh", W1r, a1_src)
    w1d = np.einsum("khc,hc->kh", W1r, a1_dst)
    W1e = np.concatenate([W1, w1s, w1d], axis=1).astype(bf16)
    w2s = W2 @ a2_src[0]
    w2d = W2 @ a2_dst[0]
    W2e = np.concatenate([W2, w2s[:, None], w2d[:, None]], axis=1).astype(bf16)
    return W1e, W2e


_CACHE = {}


def prepare(inputs, cfg=DEFAULT_CFG, reps=1):
    x = np.asarray(inputs["x"], np.float32)
    edge_index = np.asarray(inputs["edge_index"])
    W1 = np.asarray(inputs["W1"], np.float32)
    W2 = np.asarray(inputs["W2"], np.float32)
    b1 = np.asarray(inputs["b1"], np.float32)
    b2 = np.asarray(inputs["b2"], np.float32)
    a1s = np.asarray(inputs["a1_src"], np.float32)
    a1d = np.asarray(inputs["a1_dst"], np.float32)
    a2s = np.asarray(inputs["a2_src"], np.float32)
    a2d = np.asarray(inputs["a2_dst"], np.float32)

    meta1, meta2, per_core_idx = preprocess(edge_index, cfg)
    key = (cfg["N"], meta1.tiles.tobytes(), meta2.tiles.tobytes())
    if key not in _CACHE:
        nc = build_nc(cfg, meta1, meta2)
        _CACHE[key] = (nc, SpmdRunner(nc, cfg["ncores"]))
    nc, runner = _CACHE[key]

    W1e, W2e = fold_weights(W1, a1s, a1d, W2, a2s, a2d, cfg)
    b1rep = np.tile(b1[None, :], (P, 1)).astype(np.float32)
    b2rep = np.tile(b2[None, :], (P, 1)).astype(np.float32)
    nloc = cfg["nloc"]
    in_maps = []
    for c in range(cfg["ncores"]):
        m = dict(per_core_idx[c])
        m["xT"] = np.ascontiguousarray(
            x[c * nloc:(c + 1) * nloc, :].T).astype(bf16)
        m["W1e"], m["W2e"] = W1e, W2e
        m["b1r"], m["b2r"] = b1rep, b2rep
        in_maps.append(m)
    return runner, in_maps


def kernel_timed(inputs, reps=1):
    cfg = DEFAULT_CFG
    runner, in_maps = prepare(inputs, cfg, reps)
    results, best_ns = runner.run(in_maps, reps=reps)
    out = np.concatenate([results[c]["out"] for c in range(cfg["ncores"])],
                         axis=0)
    return out, best_ns


def kernel(**inputs):
    out, _ = kernel_timed(inputs, reps=1)
    return out
